# revision 4
# baseline (speedup 1.0000x reference)
"""
CrossAttention kernel for 8x Trainium2 NeuronCores (Bass/Tile).

Problem: nn_CrossAttention (B=4, Sq=Skv=1024, DM=1024, H=16, DK=64), fp32 I/O.

Sharding (Megatron-style hybrid, 8 cores = 2 batch-pairs x 4 head-quarters):
  core c -> batches {2*(c//4), 2*(c//4)+1}, heads {4*(c%4) .. 4*(c%4)+3}.
Each core computes Q/K/V projections for its 4 heads (256 of the 1024
Wq/Wk/Wv output dims), attention for those heads, and a partial O
projection (its 256 rows of Wo).  The host sums the 4 head-quarter
partials per batch and adds bo + bv@Wo (bias-after-allreduce; bv is
dropped on device since softmax weights sum to 1).

Device-side dataflow (per core), all matmuls bf16 with fp32 PSUM accumulate:
  - Host pre-transposes y1/y2 (feature-major yT, cast bf16) and ships
    ebT = exp(attn_bias)^T bf16 [HC, Skv, Sq]; no on-device casts needed.
  - qT/kT produced feature-major [dk, Sq]; bq (pre-scaled by 1/sqrt(dk)) and
    bk fused into the PSUM->SBUF evacuation via ACT activation bias.
  - v produced token-major [Skv, dk]; a ones-column is appended so the PV
    matmul emits the softmax denominator as PSUM row 64 for free (M=65).
  - logits computed transposed l^T [Skv, Sq] = k^T q; exp on ACT straight to
    bf16 in SBUF (no max-subtraction: |logits| <= ~15 is exact-exp-safe);
    the bias enters multiplicatively: p~ = exp(l) * exp(bias) via a DVE
    scalar_tensor_tensor multiply (4x mode: all-SBUF packed bf16).
  - attn^T = v^T p~^T (unnormalized) + denominator row; normalization applied
    at PSUM evacuation: r = 1/s broadcast across partitions via a DRAM
    round-trip DMA (0-step partition AP), DVE multiply during evacuation.
  - O projection token-major from assembled attn^T head-pair tiles.
"""

import os
import sys

sys.path.insert(0, "/opt/trn_rl_repo")

from contextlib import ExitStack

import numpy as np
import ml_dtypes

import concourse.bass as bass
import concourse.mybir as mybir
import concourse.tile as tile
from concourse import bacc
from concourse.bass_utils import run_bass_kernel_spmd

BF16 = mybir.dt.bfloat16
F32 = mybir.dt.float32
AF = mybir.ActivationFunctionType
ALU = mybir.AluOpType

B, Sq, Skv, DM = 4, 1024, 1024, 1024
H, DK = 16, 64
HC = 4            # heads per core
DH = HC * DK      # head dims per core (256)
BC = 2            # batches per core
SCALE = DK ** -0.5
N_CORES = 8

_PROGRAM = None   # cached (nc, out_name)
TRACE = bool(int(os.environ.get("BASS_KERNEL_TRACE", "0")))
LPS_BUFS = int(os.environ.get("K_LPS_BUFS", "2"))
APS_BUFS = int(os.environ.get("K_APS_BUFS", "4"))
OPS_BUFS = int(os.environ.get("K_OPS_BUFS", "2"))
MULT_POOL = int(os.environ.get("K_MULT_POOL", "0"))  # every Nth exp-mult on gpsimd
LAST_RESULTS = None


def build_program():
    """Build the per-core SPMD Bass program (identical on all 8 cores)."""
    nc = bacc.Bacc(
        "TRN2",
        target_bir_lowering=False,
        debug=False,
        num_devices=N_CORES,
    )

    # ---- DRAM parameters (per-core shards, host-prepared) ----
    y1T = nc.dram_tensor("y1T", [BC, DM, Skv], BF16, kind="ExternalInput")
    y2T = nc.dram_tensor("y2T", [BC, DM, Sq], BF16, kind="ExternalInput")
    ebT = nc.dram_tensor("ebT", [HC, Skv, Sq], BF16, kind="ExternalInput")
    wq = nc.dram_tensor("wq", [DM, DH], BF16, kind="ExternalInput")
    wk = nc.dram_tensor("wk", [DM, DH], BF16, kind="ExternalInput")
    wv = nc.dram_tensor("wv", [DM, DH], BF16, kind="ExternalInput")
    wo = nc.dram_tensor("wo", [DH, DM], BF16, kind="ExternalInput")
    bqv = nc.dram_tensor("bqv", [128, 2], F32, kind="ExternalInput")   # bq*SCALE, col-chunked
    bkv = nc.dram_tensor("bkv", [128, 2], F32, kind="ExternalInput")   # bk, col-chunked
    out = nc.dram_tensor("out", [BC, Sq, DM], BF16, kind="ExternalOutput")

    with tile.TileContext(nc) as tc, ExitStack() as ctx:
        build_kernel(ctx, tc, y1T, y2T, ebT, wq, wk, wv, wo, bqv, bkv, out)

    nc.compile()
    return nc, "out"


def build_kernel(ctx, tc, y1T, y2T, ebT, wq, wk, wv, wo, bqv, bkv, out):
    nc = tc.nc
    KT = DM // 128            # 8 contraction tiles for projections
    KG = 2                    # k-tiles per y DMA group
    NG = KT // KG             # 4 groups
    NQ = Sq // 512            # 2 moving-dim halves
    MS = Skv // 128           # 8 skv row tiles

    # ---------------- constant / weight loads ----------------
    consts = ctx.enter_context(tc.tile_pool(name="consts", bufs=1))

    wq_sb = consts.tile([128, KT, DH], BF16, tag="wq", name="wq")
    wk_sb = consts.tile([128, KT, DH], BF16, tag="wk", name="wk")
    wv_sb = consts.tile([128, KT, DH], BF16, tag="wv", name="wv")
    wo_sb = consts.tile([128, 2, DM], BF16, tag="wo", name="wo")   # [256,1024] -> 2 k-tiles
    for w_sb_, w_ in ((wq_sb, wq), (wk_sb, wk), (wv_sb, wv)):
        nc.sync.dma_start(
            out=w_sb_[:], in_=w_.ap().rearrange("(k p) m -> p k m", p=128)
        )
    nc.sync.dma_start(out=wo_sb[:], in_=wo.ap().rearrange("(k p) m -> p k m", p=128))

    bq_sb = consts.tile([128, 2], F32, tag="bq", name="bq")
    nc.sync.dma_start(out=bq_sb[:], in_=bqv[:, :])
    bk_sb = consts.tile([128, 2], F32, tag="bk", name="bk")
    nc.sync.dma_start(out=bk_sb[:], in_=bkv[:, :])

    # persistent activations (both batches)
    acts = ctx.enter_context(tc.tile_pool(name="acts", bufs=1))
    qT_sb = [[acts.tile([128, Sq], BF16, tag=f"qT{b}{p}", name=f"qT{b}{p}") for p in range(2)]
             for b in range(BC)]                       # [b][head-pair] rows=2x64 dk
    kT_sb = [[acts.tile([128, Skv], BF16, tag=f"kT{b}{p}", name=f"kT{b}{p}") for p in range(2)]
             for b in range(BC)]
    v_sb = [[acts.tile([128, HC, 65], BF16, tag=f"v{b}{m}", name=f"v{b}{m}") for m in range(MS)]
            for b in range(BC)]                        # [b][skv-tile]: per-head [64 v | 1 ones]
    pair_sb = [[acts.tile([128, Sq], BF16, tag=f"at{b}{p}", name=f"at{b}{p}") for p in range(2)]
               for b in range(BC)]                     # attnT head-pair tiles

    # ---------------- Phase P: Q/K projections (V deferred into step 0) ----------------
    y1g_all = {}
    ybf1 = ctx.enter_context(tc.tile_pool(name="ybf1", bufs=BC * NG))
    with ExitStack() as pctx:
        YB = int(os.environ.get("K_YBF_BUFS", "5"))
        ybf = pctx.enter_context(tc.tile_pool(name="ybf", bufs=YB))
        ppsum = pctx.enter_context(tc.tile_pool(name="ppsum", bufs=4, space="PSUM"))

        for b in range(BC):
            # --- load y2T[b] (bf16 from host), then Q projection ---
            y2g = []
            for g in range(NG):
                t = ybf.tile([128, KG, Sq], BF16, tag="ybf", name="ybf")
                nc.sync.dma_start(
                    out=t[:],
                    in_=y2T[b, 128 * KG * g:128 * KG * (g + 1), :]
                    .rearrange("(k p) q -> p k q", p=128),
                )
                y2g.append(t)
            qps = {(m, n): ppsum.tile([128, 512], F32, tag="ps", name="ps")
                   for m in range(2) for n in range(NQ)}
            for k in range(KT):           # k-outer: start as soon as group lands
                for m in range(2):
                    for n in range(NQ):
                        nc.tensor.matmul(
                            qps[(m, n)][:],
                            lhsT=wq_sb[:, k, 128 * m:128 * (m + 1)],
                            rhs=y2g[k // KG][:, k % KG, 512 * n:512 * (n + 1)],
                            start=(k == 0), stop=(k == KT - 1),
                        )
            for m in range(2):
                for n in range(NQ):
                    nc.scalar.activation(
                        qT_sb[b][m][:, 512 * n:512 * (n + 1)], qps[(m, n)][:],
                        AF.Identity, bias=bq_sb[:, m:m + 1], scale=SCALE,
                    )
            # --- load y1T[b] (persistent groups), then K projection ---
            y1g = []
            for g in range(NG):
                t = ybf1.tile([128, KG, Skv], BF16, tag="ybf1", name="ybf1")
                nc.sync.dma_start(
                    out=t[:],
                    in_=y1T[b, 128 * KG * g:128 * KG * (g + 1), :]
                    .rearrange("(k p) q -> p k q", p=128),
                )
                y1g.append(t)
            y1g_all[b] = y1g
            kps = {(m, n): ppsum.tile([128, 512], F32, tag="ps", name="ps")
                   for m in range(2) for n in range(NQ)}
            for k in range(KT):
                for m in range(2):
                    for n in range(NQ):
                        nc.tensor.matmul(
                            kps[(m, n)][:],
                            lhsT=wk_sb[:, k, 128 * m:128 * (m + 1)],
                            rhs=y1g[k // KG][:, k % KG, 512 * n:512 * (n + 1)],
                            start=(k == 0), stop=(k == KT - 1),
                        )
            for m in range(2):
                for n in range(NQ):
                    nc.scalar.activation(
                        kT_sb[b][m][:, 512 * n:512 * (n + 1)], kps[(m, n)][:],
                        AF.Identity, bias=bk_sb[:, m:m + 1], scale=1.0,
                    )

    # ---------------- Phase A: attention + interleaved O projection ----------------
    with ExitStack() as actx:
        EBP = int(os.environ.get("K_EB_BUFS", "4"))
        ebpool = actx.enter_context(tc.tile_pool(name="eb", bufs=EBP))
        lpsum = actx.enter_context(tc.tile_pool(name="lpsum", bufs=LPS_BUFS, space="PSUM"))
        apsum = actx.enter_context(tc.tile_pool(name="apsum", bufs=APS_BUFS, space="PSUM"))
        PPOOL_BUFS = int(os.environ.get("K_PPOOL_BUFS", "36"))
        ppool = actx.enter_context(tc.tile_pool(name="pT", bufs=PPOOL_BUFS))
        NP = int(os.environ.get("K_NORM_BUFS", "4"))
        npool = actx.enter_context(tc.tile_pool(name="norm", bufs=NP))
        rdram = actx.enter_context(tc.tile_pool(name="rdram", bufs=int(os.environ.get("K_RD_BUFS", "4")), space="DRAM"))
        opsum = actx.enter_context(tc.tile_pool(name="opsum", bufs=OPS_BUFS, space="PSUM"))
        opool = actx.enter_context(tc.tile_pool(name="osb", bufs=3))

        def emit_pv(b, h2, st, aps, kt):
            n_p, hp_p, pT_p = st
            h = 2 * hp_p + h2
            nc.tensor.matmul(
                aps[0:65, :],
                lhsT=v_sb[b][kt][:, h, :],
                rhs=pT_p[(b, kt, h2)][:],
                start=(kt == 0), stop=(kt == MS - 1),
            )

        def finalize_pv(b, h2, st, aps):
            n_p, hp_p, _ = st
            # r = 1/s (s = PSUM row 64, same base partition), then broadcast
            # across 64 partitions via DRAM round-trip (0-step partition AP)
            r_t = npool.tile([128, 512], F32, tag="r", name="r")
            nc.vector.reciprocal(r_t[64:65, :], aps[64:65, :])
            rd = rdram.tile([1, 512], F32, tag="rd", name="rd")
            nc.sync.dma_start(out=rd[:], in_=r_t[64:65, :])
            rd_ap = rd[:]
            rd_bcast = bass.AP(
                tensor=rd_ap.tensor,
                offset=rd_ap.offset,
                ap=[[0, 64], list(rd_ap.ap[-1])],
            )
            R_t = npool.tile([64, 512], F32, tag="R", name="R")
            nc.gpsimd.dma_start(out=R_t[:], in_=rd_bcast)
            dst = pair_sb[b][hp_p][64 * h2:64 * (h2 + 1),
                                   512 * n_p:512 * (n_p + 1)]
            if h2 == 0:
                nc.vector.scalar_tensor_tensor(
                    dst, aps[0:64, :], 1.0, R_t[:],
                    op0=ALU.mult, op1=ALU.mult,
                )
            else:
                tmp = npool.tile([64, 512], BF16, tag="atmp", name="atmp")
                nc.vector.scalar_tensor_tensor(
                    tmp[:], aps[0:64, :], 1.0, R_t[:],
                    op0=ALU.mult, op1=ALU.mult,
                )
                nc.sync.dma_start(out=dst, in_=tmp[:])

        def emit_o_half(n, only_b=None):
            # O projection for sq half n (overlaps the next attention step)
            for b in ((only_b,) if only_b is not None else range(BC)):
                for mt in range(4 * n, 4 * (n + 1)):
                    o_t = opool.tile([128, DM], BF16, tag="osb", name="osb")
                    for no in range(DM // 512):
                        ps = opsum.tile([128, 512], F32, tag="o", name="o")
                        for kp in range(2):
                            nc.tensor.matmul(
                                ps[:],
                                lhsT=pair_sb[b][kp][:, 128 * mt:128 * (mt + 1)],
                                rhs=wo_sb[:, kp, 512 * no:512 * (no + 1)],
                                start=(kp == 0), stop=(kp == 1),
                            )
                        nc.vector.tensor_copy(o_t[:, 512 * no:512 * (no + 1)], ps[:])
                    nc.sync.dma_start(
                        out=out[b, 128 * mt:128 * (mt + 1), :],
                        in_=o_t[:],
                    )

        # per step (n, hp): logits for all (b, kt, h2); exp; bias-multiply;
        # then PV of the previous step interleaves with the next step's QK.
        steps = [(n, hp) for n in range(NQ) for hp in range(2)]
        prev = None            # (n, hp, pT) of the previous step
        for si, step in enumerate(steps + [None]):
            n, hp = step if step is not None else (None, None)
            if si < len(steps):
                # exp(bias) tiles for this step's two heads (shared by b=0,1)
                eb_t = {}
                for h2 in range(2):
                    ebf = ebpool.tile([128, MS, 512], BF16, tag="eb", name="eb")
                    nc.sync.dma_start(
                        out=ebf[:],
                        in_=ebT[2 * hp + h2, :, 512 * n:512 * (n + 1)]
                        .rearrange("(k p) q -> p k q", p=128),
                    )
                    eb_t[h2] = ebf
                pT = {}
                mi = 0
                for b in range(BC):
                    for kt in range(MS):
                        for h2 in range(2):
                            lps = lpsum.tile([128, 512], F32, tag="l", name="l")
                            nc.tensor.matmul(
                                lps[:],
                                lhsT=kT_sb[b][hp][64 * h2:64 * (h2 + 1),
                                                  128 * kt:128 * (kt + 1)],
                                rhs=qT_sb[b][hp][64 * h2:64 * (h2 + 1),
                                                 512 * n:512 * (n + 1)],
                                start=True, stop=True,
                            )
                            pt = ppool.tile([128, 512], BF16, tag="pT", name="pT")
                            nc.scalar.activation(pt[:], lps[:], AF.Exp)
                            mi += 1
                            eng = (nc.gpsimd if (MULT_POOL and mi % MULT_POOL == 0)
                                   else nc.vector)
                            eng.scalar_tensor_tensor(
                                pt[:], pt[:], 1.0, eb_t[h2][:, kt, :],
                                op0=ALU.mult, op1=ALU.mult,
                            )
                            pT[(b, kt, h2)] = pt
                        if prev is not None:
                            for h2 in range(2):   # previous step's PV, same kt
                                emit_pv(b, h2, prev, aps_t[(b, h2)], kt)
                if si == 0:
                    # deferred V projections: PE work that fills step 0's
                    # exp-drain window; v is only needed from the PV block on
                    for vb in range(BC):
                        for mt in range(MS):
                            vps = opsum.tile([128, DH], F32, tag="o", name="psv")
                            for k in range(KT):
                                nc.tensor.matmul(
                                    vps[:],
                                    lhsT=y1g_all[vb][k // KG][:, k % KG,
                                                              128 * mt:128 * (mt + 1)],
                                    rhs=wv_sb[:, k, :],
                                    start=(k == 0), stop=(k == KT - 1),
                                )
                            nc.vector.tensor_copy(
                                v_sb[vb][mt][:, :, 0:64],
                                vps[:].rearrange("p (h d) -> p h d", d=DK),
                            )
                            nc.gpsimd.memset(v_sb[vb][mt][:, :, 64:65], 1.0)
            if si == len(steps) and prev is not None:
                # tail: PV for the final step (no next step to interleave with)
                for b in range(BC):
                    for kt in range(MS):
                        for h2 in range(2):
                            emit_pv(b, h2, prev, aps_t[(b, h2)], kt)
            if prev is not None:
                for b in range(BC):
                    for h2 in range(2):
                        finalize_pv(b, h2, prev, aps_t[(b, h2)])
                n_p, hp_p, _ = prev
                if hp_p == 1:
                    emit_o_half(n_p)
            if si < len(steps):
                prev = (n, hp, pT)
                aps_t = {(b, h2): apsum.tile([128, 512], F32, tag="av", name="av")
                         for b in range(BC) for h2 in range(2)}
            else:
                prev = None

# ====================== host wrapper ======================

def _prep_core_inputs(c, y1, y2, attn_bias, Wq, bq, Wk, bk, Wv, bv, Wo, bo):
    bp, hq = c // 4, c % 4
    bsl = slice(2 * bp, 2 * bp + 2)
    hsl = slice(DH * hq, DH * (hq + 1))
    bf16 = ml_dtypes.bfloat16
    f32 = np.float32
    eb = np.exp(attn_bias[0, 4 * hq:4 * hq + 4].astype(f32))
    return {
        "y1T": np.ascontiguousarray(y1[bsl].transpose(0, 2, 1)).astype(bf16),
        "y2T": np.ascontiguousarray(y2[bsl].transpose(0, 2, 1)).astype(bf16),
        "ebT": np.ascontiguousarray(eb.transpose(0, 2, 1)).astype(bf16),
        "wq": np.ascontiguousarray(Wq[:, hsl]).astype(bf16),
        "wk": np.ascontiguousarray(Wk[:, hsl]).astype(bf16),
        "wv": np.ascontiguousarray(Wv[:, hsl]).astype(bf16),
        "wo": np.ascontiguousarray(Wo[hsl, :]).astype(bf16),
        "bqv": np.ascontiguousarray(
            (bq[hsl].astype(f32) * SCALE).reshape(2, 128).T
        ),
        "bkv": np.ascontiguousarray(bk[hsl].astype(f32).reshape(2, 128).T),
    }


def kernel(y1, y2, attn_bias, Wq, bq, Wk, bk, Wv, bv, Wo, bo):
    global _PROGRAM, LAST_RESULTS
    args = [np.asarray(x) for x in
            (y1, y2, attn_bias, Wq, bq, Wk, bk, Wv, bv, Wo, bo)]
    if _PROGRAM is None:
        _PROGRAM = build_program()
    nc, out_name = _PROGRAM

    in_maps = [_prep_core_inputs(c, *args) for c in range(N_CORES)]
    res = run_bass_kernel_spmd(nc, in_maps, list(range(N_CORES)), trace=TRACE)
    LAST_RESULTS = res

    out = np.zeros((B, Sq, DM), np.float32)
    for c in range(N_CORES):
        part = np.asarray(res.results[c][out_name]).astype(np.float32)
        bp = c // 4
        out[2 * bp] += part[0]
        out[2 * bp + 1] += part[1]
    # bias after partial-sum: bo plus the folded v-bias contribution bv @ Wo
    bias_full = (args[10].astype(np.float32)
                 + args[8].astype(np.float32) @ args[9].astype(np.float32))
    out += bias_full[None, None, :]
    return out
